# revision 9
# baseline (speedup 1.0000x reference)
"""AdaptiveFanOut kernel for 8 TRN2 NeuronCores.

out[b, cumsum(counts[b])[i]-1] += hidden[b, i]  on top of residual[b].

Valid counts are >= 1, so scatter targets are strictly increasing per
batch row -> no collisions -> the scatter-add is a pure row scatter.  We
invert it on the host into a gather map g (output row j <- hidden row
g[j], OOB sentinel for "no contribution") and run, per core (one batch
row per core), a pure-DMA pipeline over 4 blocks of 1024 output rows.

Within a block, output row j = block_start + p*8 + s lives in SBUF
partition p, free-dim slot s.  That makes the residual load / out store
one 4 MB DMA with 128 contiguous 32 KB descriptors, while the hidden
contribution is added by 8 indirect gather DMAs (one per slot s), each
carrying one int32 row index per partition, with cce add and
out-of-bounds indices silently skipped (-> masked rows cost nothing).

HBM traffic per core: 16 MB residual + 8 MB hidden + 16 MB out = 40 MB,
the memory roofline for this problem.
"""

import numpy as np

B, NEW_S, RES_S, H = 8, 2048, 4096, 1024
BLOCK = 1024            # output rows per pipeline block
S = BLOCK // 128        # free-dim slots per partition (8)
NBLOCK = RES_S // BLOCK  # 4
OOB = NEW_S             # gather sentinel (> bounds_check -> skipped)

_cached = {}


def _build_program(repeat: int = 1):
    import concourse.bacc as bacc
    import concourse.bass as bass
    import concourse.mybir as mybir
    import concourse.tile as tile

    f32 = mybir.dt.float32
    i32 = mybir.dt.int32

    nc = bacc.Bacc("TRN2", target_bir_lowering=False, debug=False)
    hidden = nc.dram_tensor("hidden", [NEW_S, H], f32, kind="ExternalInput")
    residual = nc.dram_tensor("residual", [RES_S, H], f32, kind="ExternalInput")
    gidx = nc.dram_tensor("gidx", [128, S * NBLOCK], i32, kind="ExternalInput")
    out = nc.dram_tensor("out", [RES_S, H], f32, kind="ExternalOutput")

    with tile.TileContext(nc) as tc:
        with (
            tc.tile_pool(name="ipool", bufs=1) as ipool,
            tc.tile_pool(name="tpool", bufs=3) as tpool,
        ):
            gi = ipool.tile([128, S * NBLOCK], i32)
            nc.sync.dma_start(out=gi[:], in_=gidx[:])
            for _ in range(repeat):
                for c in range(NBLOCK):
                    T = tpool.tile([128, S * H], f32)
                    rview = residual[c * BLOCK : (c + 1) * BLOCK].rearrange(
                        "(p s) h -> p (s h)", p=128
                    )
                    nc.sync.dma_start(out=T[:], in_=rview)
                    for s in range(S):
                        nc.gpsimd.indirect_dma_start(
                            out=T[:, s * H : (s + 1) * H],
                            out_offset=None,
                            in_=hidden[:],
                            in_offset=bass.IndirectOffsetOnAxis(
                                ap=gi[:, c * S + s : c * S + s + 1], axis=0
                            ),
                            bounds_check=NEW_S - 1,
                            oob_is_err=False,
                            compute_op=mybir.AluOpType.add,
                        )
                    oview = out[c * BLOCK : (c + 1) * BLOCK].rearrange(
                        "(p s) h -> p (s h)", p=128
                    )
                    nc.sync.dma_start(out=oview, in_=T[:])
    return nc


def _build_bench_program_static(repeat: int):
    """Timing variant: hidden/residual/out are internal DRAM tensors (no
    host I/O), body statically unrolled `repeat` times (0 = prologue
    only). Only gidx is a real input; output is a tiny dummy."""
    import concourse.bacc as bacc
    import concourse.bass as bass
    import concourse.mybir as mybir
    import concourse.tile as tile

    f32 = mybir.dt.float32
    i32 = mybir.dt.int32

    nc = bacc.Bacc("TRN2", target_bir_lowering=False, debug=False)
    hidden = nc.dram_tensor("hidden_i", [NEW_S, H], f32)
    residual = nc.dram_tensor("residual_i", [RES_S, H], f32)
    out = nc.dram_tensor("out_i", [RES_S, H], f32)
    gidx = nc.dram_tensor("gidx", [128, S * NBLOCK], i32, kind="ExternalInput")
    dummy = nc.dram_tensor("bench_out", [128, 4], i32, kind="ExternalOutput")

    with tile.TileContext(nc) as tc:
        with (
            tc.tile_pool(name="ipool", bufs=1) as ipool,
            tc.tile_pool(name="tpool", bufs=3) as tpool,
        ):
            gi = ipool.tile([128, S * NBLOCK], i32)
            nc.sync.dma_start(out=gi[:], in_=gidx[:])
            Z = ipool.tile([128, S * H], f32)
            nc.gpsimd.memset(Z[:], 0.0)
            for c in range(NBLOCK):
                zv = residual[c * BLOCK : (c + 1) * BLOCK].rearrange(
                    "(p s) h -> p (s h)", p=128
                )
                nc.sync.dma_start(out=zv, in_=Z[:])
            for c in range(NBLOCK // 2):
                zv = hidden[c * BLOCK : (c + 1) * BLOCK].rearrange(
                    "(p s) h -> p (s h)", p=128
                )
                nc.sync.dma_start(out=zv, in_=Z[:])
            for _ in range(repeat):
                for c in range(NBLOCK):
                    T = tpool.tile([128, S * H], f32)
                    rview = residual[c * BLOCK : (c + 1) * BLOCK].rearrange(
                        "(p s) h -> p (s h)", p=128
                    )
                    nc.sync.dma_start(out=T[:], in_=rview)
                    for s in range(S):
                        nc.gpsimd.indirect_dma_start(
                            out=T[:, s * H : (s + 1) * H],
                            out_offset=None,
                            in_=hidden[:],
                            in_offset=bass.IndirectOffsetOnAxis(
                                ap=gi[:, c * S + s : c * S + s + 1], axis=0
                            ),
                            bounds_check=NEW_S - 1,
                            oob_is_err=False,
                            compute_op=mybir.AluOpType.add,
                        )
                    oview = out[c * BLOCK : (c + 1) * BLOCK].rearrange(
                        "(p s) h -> p (s h)", p=128
                    )
                    nc.sync.dma_start(out=oview, in_=T[:])
            nc.sync.dma_start(out=dummy[:], in_=gi[:, :4])
    return nc


def get_program():
    if "nc" not in _cached:
        nc = _build_program()
        nc.finalize()
        _cached["nc"] = nc
    return _cached["nc"]


def make_gather_map(counts_row: np.ndarray) -> np.ndarray:
    """counts_row: (NEW_S,) int. Returns g: (RES_S,) int32 where
    out[j] = residual[j] + (g[j] < NEW_S ? hidden[g[j]] : 0)."""
    c = np.asarray(counts_row).astype(np.int64)
    valid = np.cumprod(c > 0) > 0          # break at first zero count
    idx = np.cumsum(c) - 1                 # scatter target per source row
    g = np.full(RES_S, OOB, dtype=np.int32)
    src = np.nonzero(valid)[0]
    tgt = idx[valid]
    keep = tgt < RES_S                     # mode='drop' semantics
    g[tgt[keep]] = src[keep].astype(np.int32)
    return g


def make_in_maps(hidden_states, counts, residual_states):
    in_maps = []
    for b in range(B):
        g = make_gather_map(counts[b])
        # device layout: block c, partition p, slot s <-> row c*BLOCK + p*S + s
        # gidx[p, c*S + s] = g[c*BLOCK + p*S + s]
        gtile = (
            g.reshape(NBLOCK, 128, S).transpose(1, 0, 2).reshape(128, NBLOCK * S)
        )
        in_maps.append(
            {
                "hidden": np.ascontiguousarray(hidden_states[b], dtype=np.float32),
                "residual": np.ascontiguousarray(
                    residual_states[b], dtype=np.float32
                ),
                "gidx": np.ascontiguousarray(gtile),
            }
        )
    return in_maps


def kernel(**inputs) -> np.ndarray:
    from concourse.bass_utils import run_bass_kernel_spmd

    hidden_states = np.asarray(inputs["hidden_states"], dtype=np.float32)
    counts = np.asarray(inputs["merged_embeddings_counts"])
    residual_states = np.asarray(inputs["residual_hidden_states"], dtype=np.float32)

    nc = get_program()
    in_maps = make_in_maps(hidden_states, counts, residual_states)
    res = run_bass_kernel_spmd(nc, in_maps, core_ids=list(range(B)))
    return np.stack([np.asarray(res.results[i]["out"]) for i in range(B)])
